# revision 37
# baseline (speedup 1.0000x reference)
"""MoE (top-2 of 8 experts) Trainium2 kernel.

Strategy (expert-parallel over 8 NeuronCores):
  - Router (x @ Wr -> softmax -> top-2 -> renormalize) runs on host: it is
    ~0.1% of total FLOPs and produces the token->expert dispatch that defines
    the sharding itself.
  - Each core e receives the tokens routed to expert e (gathered, transposed
    to [D, C], zero-padded to capacity C) plus expert e's weights, and runs
    the 3-layer MLP fully on-device in a transposed dataflow:
        h1T = relu(W1^T x^T + b1)   [H,  C]
        h2T = relu(W2^T h1T + b2)   [H2, C]
        yT  = W3^T h2T + b3         [O,  C]
    All matmul contractions sit on the partition axis, so no on-chip
    transposes are needed anywhere.
  - Host combines per-expert outputs with the renormalized top-2 routing
    weights (scatter-add), exactly matching the reference's dense-combine
    semantics.
  - Matmuls run in bf16 with fp32 PSUM accumulation (measured ~4e-3 max
    relative error vs the fp32 reference).
  - If any expert receives more than C tokens (never observed; capacity is
    1.25x the expected per-expert load), the leftover tokens are processed in
    an additional run of the same NEFF - correctness never depends on C.
"""

import re as _re

import numpy as np
import ml_dtypes

import bass_rust as _bass_rust
import concourse.bass as bass
import concourse.mybir as mybir
import concourse.tile as tile
from concourse.bass_utils import run_bass_kernel_spmd


def _split_drain_and_barrier(self, tick_clock, wait_clock):
    """Replacement for TileContext._drain_and_barrier.

    The stock version hangs every outstanding proc semaphore wait on one
    Drain instruction; the walrus in this environment rejects any
    instruction carrying more than one sync wait. Emit the same waits as
    individual sync-engine wait_ge instructions (one wait each) before a
    clean drain instead.
    """
    ticks = [
        int(v)
        for v in _re.findall(r"\d+", repr(tick_clock.global_clock))
    ]
    for proc, sem in sorted(self.sems.allocated().items()):
        if proc < len(ticks) and ticks[proc] > 0:
            self.nc.sync.wait_ge(sem, _bass_rust.tick_to_sem(ticks[proc], proc))
    self.nc.sync.drain()

    self.nc.all_engine_barrier()
    assert self.sems is not None
    popped = self.nc._tile_sem_poison_stack.pop()
    assert popped is self._sem_poison
    self.nc.clear_and_free_semaphores(list(self.sems.allocated().values()))
    self.nc.all_engine_barrier()


tile.TileContext._drain_and_barrier = _split_drain_and_barrier

B, D, H, E, O, TOP_K = 8192, 1024, 2048, 8, 10, 2
H2 = H // 2
NCORES = 8
P = 128

TWS = [512, 512, 512, 512, 384]   # token tile widths (<=512 = one PSUM bank)
C = sum(TWS)      # per-expert token capacity (tokens, padded)
NT = len(TWS)     # token tiles
KD = D // P       # 8   k-chunks for layer 1
MH = H // P       # 16  m-tiles for layer 1 / k-chunks for layer 2
MH2 = H2 // P     # 8   m-tiles for layer 2 / k-chunks for layer 3

BF16 = mybir.dt.bfloat16
F32 = mybir.dt.float32
_nbf16 = ml_dtypes.bfloat16


NW1 = KD * H          # w1 columns in the packed weight tile
NW2 = MH * H2         # w2 columns
NW3 = MH2 * O         # w3 columns
NWTOT = NW1 + NW2 + NW3


def _build_nc(with_bias: bool) -> bass.Bass:
    nc = bass.Bass()
    # Host pre-packs everything into the on-chip layout:
    #  xt   [128, KD, C]  — x gathered/transposed, k-chunks on axis 1
    #  w1/w2/w3 packed k-chunk-major: [128, KD*H] etc., bf16
    xt = nc.dram_tensor("xt", [P, KD, C], BF16, kind="ExternalInput")
    w1d = nc.dram_tensor("w1p", [P, NW1], BF16, kind="ExternalInput")
    w2d = nc.dram_tensor("w2p", [P, NW2], BF16, kind="ExternalInput")
    w3d = nc.dram_tensor("w3p", [P, NW3], BF16, kind="ExternalInput")
    if with_bias:
        # biases as single-partition rows, pre-cast to bf16 on host:
        # cols [0,H) = b1, [H,H+H2) = b2, [H+H2,H+H2+O) = b3
        bias = nc.dram_tensor("bias", [1, H + H2 + O], BF16, kind="ExternalInput")
    # Layer 3 runs as 4 concurrent PE column-group matmuls (2 accumulation
    # rounds of the 8 k-chunks); partials live at psum partitions 32j..32j+O
    # and the host sums the four groups.
    out = nc.dram_tensor("out", [3 * 32 + O, C], F32, kind="ExternalOutput")

    relu_kw = dict(op0=mybir.AluOpType.max)

    with tile.TileContext(nc) as tc:
        with (
            tc.tile_pool(name="weights", bufs=1) as wpool,
            tc.tile_pool(name="xin", bufs=1) as xpool,
            tc.tile_pool(name="ps1", bufs=2, space="PSUM") as ps1pool,
            tc.tile_pool(name="ps2", bufs=2, space="PSUM") as ps2pool,
            tc.tile_pool(name="ps3", bufs=2, space="PSUM") as ps3pool,
            tc.tile_pool(name="acts", bufs=2) as apool,
        ):
            # The first x tile and w1 gate the first matmul group. Chain
            # every other input DMA behind them (each chained DMA carries
            # exactly one wait) so the critical transfers get the full
            # HBM bandwidth instead of sharing it 8 ways.
            xsb_tiles = []
            off = 0
            dma_chain = []
            xsb = xpool.tile([P, KD, TWS[0]], BF16, tag="x0")
            dma_chain.append(nc.sync.dma_start(xsb, xt[:, :, 0:TWS[0]]))
            xsb_tiles.append(xsb)
            off = TWS[0]
            w1sb = wpool.tile([P, NW1], BF16)
            dma_chain.append(nc.sync.dma_start(w1sb, w1d[:, :]))
            w2sb = wpool.tile([P, NW2], BF16)
            dma_chain.append(nc.sync.dma_start(w2sb, w2d[:, :]))
            for t, tw in list(enumerate(TWS))[1:]:
                xsb = xpool.tile([P, KD, tw], BF16, tag=f"x{t}")
                dma_chain.append(nc.sync.dma_start(xsb, xt[:, :, off:off + tw]))
                xsb_tiles.append(xsb)
                off += tw
            w3sb = wpool.tile([P, NW3], BF16)
            dma_chain.append(nc.sync.dma_start(w3sb, w3d[:, :]))
            for i in range(1, len(dma_chain)):
                tile.add_dep_helper(
                    dma_chain[i].ins, dma_chain[i - 1].ins,
                    reason="serialize input DMAs: critical-path first",
                )

            def w1s(k, m):
                off = k * H + m * P
                return w1sb[:, off:off + P]

            def w2s(k, m):
                off = k * H2 + m * P
                return w2sb[:, off:off + P]

            def w3s(k):
                off = k * O
                return w3sb[:, off:off + O]

            if with_bias:
                # Bias folded into each accumulation group as one extra K=1
                # matmul against a ones row: psum[m, n] += b[m] * 1. This
                # keeps bias handling entirely on the PE, so no evacuation
                # instruction ever needs a second semaphore wait.
                bsb = wpool.tile([1, H + H2 + O], BF16)
                nc.sync.dma_start(bsb, bias[:, :])
                ones = wpool.tile([1, max(TWS)], BF16)
                nc.vector.memset(ones, 1.0)

            def bias_mm(ps, lo, hi, tw):
                if with_bias:
                    nc.tensor.matmul(
                        ps, bsb[:, lo:hi], ones[:, :tw], start=False, stop=True
                    )

            # 1-element DVE reads of the previous tile's activation buffers.
            # Slot reuse makes the first evacuation of a tile WAW-depend on
            # the previous tile's writes; the fence absorbs that own-engine
            # completion wait so no evacuation needs two semaphore waits
            # (the ISA wait slot fits only one).
            fence = wpool.tile([1, 4], BF16)
            prev = {}

            def dve_fence(key, ap):
                if key in prev:
                    nc.vector.tensor_copy(fence[:, 0:1], prev[key])
                prev[key] = ap

            # Output accumulates into one SBUF tile, DMA'd out once at the
            # end (single SWDGE transfer with a single data wait).
            osb = wpool.tile([3 * 32 + O, C], F32)

            tok_off = 0
            for t, tw in enumerate(TWS):
                tok = slice(tok_off, tok_off + tw)
                tok_off += tw
                xsb = xsb_tiles[t]

                h1sb = apool.tile([P, MH, tw], BF16, tag="h1")
                dve_fence("h1", h1sb[0:1, 0, 0:1])
                for m in range(MH):
                    ps = ps1pool.tile([P, 512], F32, tag="ps1", name="ps1t")[:, :tw]
                    for k in range(KD):
                        nc.tensor.matmul(
                            ps,
                            w1s(k, m),
                            xsb[:, k, :],
                            start=(k == 0),
                            stop=(k == KD - 1) and not with_bias,
                        )
                    bias_mm(ps, m * P, (m + 1) * P, tw)
                    nc.vector.tensor_scalar(
                        h1sb[:, m, :], ps, 0.0, None, **relu_kw
                    )

                h2sb = apool.tile([P, MH2, tw], BF16, tag="h2")
                dve_fence("h2", h2sb[0:1, 0, 0:1])
                for m in range(MH2):
                    ps = ps2pool.tile([P, 512], F32, tag="ps2", name="ps2t")[:, :tw]
                    for k in range(MH):
                        nc.tensor.matmul(
                            ps,
                            w2s(k, m),
                            h1sb[:, k, :],
                            start=(k == 0),
                            stop=(k == MH - 1) and not with_bias,
                        )
                    bias_mm(ps, H + m * P, H + (m + 1) * P, tw)
                    nc.vector.tensor_scalar(
                        h2sb[:, m, :], ps, 0.0, None, **relu_kw
                    )

                # Layer 3: the stationary is only O=10 columns wide, so pack
                # 4 k-chunks into distinct 32-wide PE column groups — they
                # run concurrently in the array. 2 accumulation rounds cover
                # all 8 chunks; group j's partial lands at psum partitions
                # 32j..32j+O and the host sums the groups.
                ps3t = ps3pool.tile([P, 512], F32, tag="ps3", name="ps3t")
                ps3 = ps3t[:, :tw]
                for r in range(2):
                    for j in range(4):
                        k = r * 4 + j
                        nc.tensor.matmul(
                            ps3[32 * j:32 * j + O, :],
                            w3s(k),
                            h2sb[:, k, :],
                            start=(r == 0),
                            stop=(r == 1) and not with_bias,
                            tile_position=(0, 32 * j),
                        )
                bias_mm(ps3[:O, :], H + H2, H + H2 + O, tw)
                nc.vector.tensor_copy(osb[:, tok], ps3[:3 * 32 + O, :])
            # SWDGE (gpsimd-issued) so the output transfer doesn't add
            # another HWDGE queue semaphore to the kernel-tail drain.
            nc.gpsimd.dma_start(out[:, :], osb)
    return nc


_NC_CACHE: dict = {}


def _get_nc(with_bias: bool) -> bass.Bass:
    if with_bias not in _NC_CACHE:
        _NC_CACHE[with_bias] = _build_nc(with_bias)
    return _NC_CACHE[with_bias]


def _route(x, Wr, br):
    """Host router: softmax over logits, top-2, renormalized weights."""
    logits = x.astype(np.float32) @ Wr.astype(np.float32) + br.astype(np.float32)
    m = logits.max(axis=-1, keepdims=True)
    p = np.exp(logits - m)
    p /= p.sum(axis=-1, keepdims=True)
    top_i = np.argsort(-p, axis=-1, kind="stable")[:, :TOP_K]
    top_p = np.take_along_axis(p, top_i, axis=-1)
    top_p = top_p / top_p.sum(axis=-1, keepdims=True)
    return top_i.astype(np.int64), top_p.astype(np.float32)


def _run_rounds(x, top_i, top_p, W1, b1, W2, b2, W3, b3, trace=False):
    """Dispatch tokens to expert-owning cores, run the NEFF, combine."""
    with_bias = bool(np.any(b1) or np.any(b2) or np.any(b3))
    nc = _get_nc(with_bias)

    # Static per-core weight inputs, packed into the on-chip layout:
    # [128 partitions, k-chunk-major columns] per weight matrix.
    w_maps = []
    for e in range(NCORES):
        m = {
            "w1p": np.ascontiguousarray(
                W1[e].reshape(KD, P, H).transpose(1, 0, 2).reshape(P, NW1)
            ).astype(_nbf16),
            "w2p": np.ascontiguousarray(
                W2[e].reshape(MH, P, H2).transpose(1, 0, 2).reshape(P, NW2)
            ).astype(_nbf16),
            "w3p": np.ascontiguousarray(
                W3[e].reshape(MH2, P, O).transpose(1, 0, 2).reshape(P, NW3)
            ).astype(_nbf16),
        }
        if with_bias:
            m["bias"] = np.concatenate(
                [b1[e], b2[e], b3[e]]
            ).reshape(1, H + H2 + O).astype(_nbf16)
        w_maps.append(m)

    # (token, slot) pairs per expert.
    tok_by_e = []
    wt_by_e = []
    for e in range(NCORES):
        tok, slot = np.nonzero(top_i == e)
        tok_by_e.append(tok)
        wt_by_e.append(top_p[tok, slot])

    out = np.zeros((B, O), np.float32)
    offset = [0] * NCORES
    last_result = None
    while True:
        active = [e for e in range(NCORES) if offset[e] < len(tok_by_e[e])]
        if not active and last_result is not None:
            break
        in_maps = []
        chunks = []
        for e in range(NCORES):
            tok = tok_by_e[e][offset[e]:offset[e] + C]
            chunks.append(tok)
            xt = np.zeros((P, KD, C), _nbf16)
            if len(tok):
                # [n, D] -> [D, n] -> k-chunks [KD, P, n] -> [P, KD, n]
                xg = x[tok].astype(_nbf16).T.reshape(KD, P, len(tok))
                xt[:, :, :len(tok)] = xg.transpose(1, 0, 2)
            in_maps.append({"xt": np.ascontiguousarray(xt), **w_maps[e]})
        res = run_bass_kernel_spmd(
            nc, in_maps, core_ids=list(range(NCORES)), trace=trace
        )
        last_result = res
        for e in range(NCORES):
            tok = chunks[e]
            if len(tok) == 0:
                continue
            oraw = res.results[e]["out"]
            y = (
                oraw[0:O, :len(tok)]
                + oraw[32:32 + O, :len(tok)]
                + oraw[64:64 + O, :len(tok)]
                + oraw[96:96 + O, :len(tok)]
            ).T  # [n_e, O]
            w = wt_by_e[e][offset[e]:offset[e] + len(tok)]
            np.add.at(out, tok, w[:, None] * y)
            offset[e] += len(tok)
    return out, last_result


def kernel(x, Wr, br, W1, b1, W2, b2, W3, b3):
    x = np.asarray(x, np.float32)
    top_i, top_p = _route(x, np.asarray(Wr), np.asarray(br))
    out, _ = _run_rounds(
        x, top_i, top_p,
        np.asarray(W1), np.asarray(b1), np.asarray(W2), np.asarray(b2),
        np.asarray(W3), np.asarray(b3),
    )
    return out


def run_traced(x, Wr, br, W1, b1, W2, b2, W3, b3):
    """Like kernel() but returns (out, BassKernelResults) with profile info."""
    x = np.asarray(x, np.float32)
    top_i, top_p = _route(x, np.asarray(Wr), np.asarray(br))
    return _run_rounds(
        x, top_i, top_p,
        np.asarray(W1), np.asarray(b1), np.asarray(W2), np.asarray(b2),
        np.asarray(W3), np.asarray(b3),
        trace=True,
    )


# revision 47
# speedup vs baseline: 1.2629x; 1.2629x over previous
"""MoE (top-2 of 8 experts) Trainium2 kernel.

Strategy (expert-parallel over 8 NeuronCores):
  - Router (x @ Wr -> softmax -> top-2 -> renormalize) runs on host: it is
    ~0.1% of total FLOPs and produces the token->expert dispatch that defines
    the sharding itself.
  - Each core e receives the tokens routed to expert e (gathered, transposed
    to [D, C], zero-padded to capacity C) plus expert e's weights, and runs
    the 3-layer MLP fully on-device in a transposed dataflow:
        h1T = relu(W1^T x^T + b1)   [H,  C]
        h2T = relu(W2^T h1T + b2)   [H2, C]
        yT  = W3^T h2T + b3         [O,  C]
    All matmul contractions sit on the partition axis, so no on-chip
    transposes are needed anywhere.
  - Host combines per-expert outputs with the renormalized top-2 routing
    weights (scatter-add), exactly matching the reference's dense-combine
    semantics.
  - Matmuls run in bf16 with fp32 PSUM accumulation (measured ~4e-3 max
    relative error vs the fp32 reference).
  - If any expert receives more than C tokens (never observed; capacity is
    1.25x the expected per-expert load), the leftover tokens are processed in
    an additional run of the same NEFF - correctness never depends on C.
"""

import re as _re

import numpy as np
import ml_dtypes

import bass_rust as _bass_rust
import concourse.bass as bass
import concourse.mybir as mybir
import concourse.tile as tile
from concourse.bass_utils import run_bass_kernel_spmd


def _split_drain_and_barrier(self, tick_clock, wait_clock):
    """Replacement for TileContext._drain_and_barrier.

    The stock version hangs every outstanding proc semaphore wait on one
    Drain instruction; the walrus in this environment rejects any
    instruction carrying more than one sync wait. Emit the same waits as
    individual sync-engine wait_ge instructions (one wait each) before a
    clean drain instead.
    """
    ticks = [
        int(v)
        for v in _re.findall(r"\d+", repr(tick_clock.global_clock))
    ]
    for proc, sem in sorted(self.sems.allocated().items()):
        if proc < len(ticks) and ticks[proc] > 0:
            self.nc.sync.wait_ge(sem, _bass_rust.tick_to_sem(ticks[proc], proc))
    self.nc.sync.drain()

    self.nc.all_engine_barrier()
    assert self.sems is not None
    popped = self.nc._tile_sem_poison_stack.pop()
    assert popped is self._sem_poison
    self.nc.clear_and_free_semaphores(list(self.sems.allocated().values()))
    self.nc.all_engine_barrier()


tile.TileContext._drain_and_barrier = _split_drain_and_barrier

B, D, H, E, O, TOP_K = 8192, 1024, 2048, 8, 10, 2
H2 = H // 2
NCORES = 8
P = 128

TWS = [512, 512, 512, 512, 384]   # token tile widths (<=512 = one PSUM bank)
C = sum(TWS)      # per-expert token capacity (tokens, padded)
NT = len(TWS)     # token tiles
KD = D // P       # 8   k-chunks for layer 1
MH = H // P       # 16  m-tiles for layer 1 / k-chunks for layer 2
MH2 = H2 // P     # 8   m-tiles for layer 2 / k-chunks for layer 3

BF16 = mybir.dt.bfloat16
F32 = mybir.dt.float32
_nbf16 = ml_dtypes.bfloat16


NW1 = KD * H          # w1 columns in the packed weight tile
NW2 = MH * H2         # w2 columns
NW3 = MH2 * O         # w3 columns
NWTOT = NW1 + NW2 + NW3
W1G = 4               # w1 arrives in this many m-major group DMAs


def _build_nc(with_bias: bool) -> bass.Bass:
    nc = bass.Bass()
    # Host pre-packs everything into the on-chip layout:
    #  xt   [128, KD, C]  — x gathered/transposed, k-chunks on axis 1
    #  w1/w2/w3 packed k-chunk-major: [128, KD*H] etc., bf16
    xt = nc.dram_tensor("xt", [P, KD, C], BF16, kind="ExternalInput")
    w1d = nc.dram_tensor("w1p", [P, NW1], BF16, kind="ExternalInput")
    w2d = nc.dram_tensor("w2p", [P, NW2], BF16, kind="ExternalInput")
    w3d = nc.dram_tensor("w3p", [P, NW3], BF16, kind="ExternalInput")
    if with_bias:
        # biases as single-partition rows, pre-cast to bf16 on host:
        # cols [0,H) = b1, [H,H+H2) = b2, [H+H2,H+H2+O) = b3
        bias = nc.dram_tensor("bias", [1, H + H2 + O], BF16, kind="ExternalInput")
    out = nc.dram_tensor("out", [O, C], F32, kind="ExternalOutput")

    relu_kw = dict(op0=mybir.AluOpType.max)

    with tile.TileContext(nc) as tc:
        with (
            tc.tile_pool(name="weights", bufs=1) as wpool,
            tc.tile_pool(name="xin", bufs=1) as xpool,
            tc.tile_pool(name="ps1", bufs=2, space="PSUM") as ps1pool,
            tc.tile_pool(name="ps2", bufs=2, space="PSUM") as ps2pool,
            tc.tile_pool(name="ps3", bufs=2, space="PSUM") as ps3pool,
            tc.tile_pool(name="acts", bufs=2) as apool,
        ):
            # w1 and the first x tile gate the first matmul group — give
            # each its own DMA (own HWDGE queue) so they stream in
            # parallel instead of behind one monolithic transfer. w1 is
            # further packed m-major and split into W1G group-DMAs with
            # separate tiles, so the m-loop's first groups start while the
            # rest of w1 is still in flight.
            MG = MH // W1G        # m-tiles per w1 group
            w1g_tiles = []
            for g in range(W1G):
                w1g = wpool.tile([P, MG * KD * P], BF16, name=f"w1g{g}")
                nc.sync.dma_start(
                    w1g, w1d[:, g * MG * KD * P:(g + 1) * MG * KD * P])
                w1g_tiles.append(w1g)
            xsb_tiles = []
            off = 0
            for t, tw in enumerate(TWS):
                xsb = xpool.tile([P, KD, tw], BF16, tag=f"x{t}")
                nc.sync.dma_start(xsb, xt[:, :, off:off + tw])
                xsb_tiles.append(xsb)
                off += tw
            w2sb = wpool.tile([P, NW2], BF16)
            nc.sync.dma_start(w2sb, w2d[:, :])
            w3sb = wpool.tile([P, NW3], BF16)
            nc.sync.dma_start(w3sb, w3d[:, :])

            def w1s(k, m):
                g, mm_ = divmod(m, MG)
                off = (mm_ * KD + k) * P
                return w1g_tiles[g][:, off:off + P]

            def w2s(k, m):
                off = k * H2 + m * P
                return w2sb[:, off:off + P]

            def w3s(k):
                off = k * O
                return w3sb[:, off:off + O]

            if with_bias:
                # Bias folded into each accumulation group as one extra K=1
                # matmul against a ones row: psum[m, n] += b[m] * 1. This
                # keeps bias handling entirely on the PE, so no evacuation
                # instruction ever needs a second semaphore wait.
                bsb = wpool.tile([1, H + H2 + O], BF16)
                nc.sync.dma_start(bsb, bias[:, :])
                ones = wpool.tile([1, max(TWS)], BF16)
                nc.vector.memset(ones, 1.0)

            def bias_mm(ps, lo, hi, tw):
                if with_bias:
                    nc.tensor.matmul(
                        ps, bsb[:, lo:hi], ones[:, :tw], start=False, stop=True
                    )

            # 1-element DVE reads of the previous tile's activation buffers.
            # Slot reuse makes the first evacuation of a tile WAW-depend on
            # the previous tile's writes; the fence absorbs that own-engine
            # completion wait so no evacuation needs two semaphore waits
            # (the ISA wait slot fits only one).
            fence = wpool.tile([1, 4], BF16)
            prev = {}

            def dve_fence(key, ap):
                if key in prev:
                    nc.vector.tensor_copy(fence[:, 0:1], prev[key])
                prev[key] = ap



            tok_off = 0
            for t, tw in enumerate(TWS):
                tok = slice(tok_off, tok_off + tw)
                tok_off += tw
                xsb = xsb_tiles[t]

                h1sb = apool.tile([P, MH, tw], BF16, tag="h1")
                dve_fence("h1", h1sb[0:1, 0, 0:1])
                for m in range(MH):
                    ps = ps1pool.tile([P, 512], F32, tag="ps1", name="ps1t")[:, :tw]
                    for k in range(KD):
                        nc.tensor.matmul(
                            ps,
                            w1s(k, m),
                            xsb[:, k, :],
                            start=(k == 0),
                            stop=(k == KD - 1) and not with_bias,
                        )
                    bias_mm(ps, m * P, (m + 1) * P, tw)
                    nc.vector.tensor_scalar(
                        h1sb[:, m, :], ps, 0.0, None, **relu_kw
                    )

                h2sb = apool.tile([P, MH2, tw], BF16, tag="h2")
                dve_fence("h2", h2sb[0:1, 0, 0:1])
                for m in range(MH2):
                    ps = ps2pool.tile([P, 512], F32, tag="ps2", name="ps2t")[:, :tw]
                    for k in range(MH):
                        nc.tensor.matmul(
                            ps,
                            w2s(k, m),
                            h1sb[:, k, :],
                            start=(k == 0),
                            stop=(k == MH - 1) and not with_bias,
                        )
                    bias_mm(ps, H + m * P, H + (m + 1) * P, tw)
                    nc.vector.tensor_scalar(
                        h2sb[:, m, :], ps, 0.0, None, **relu_kw
                    )

                ps3 = ps3pool.tile([P, 512], F32, tag="ps3", name="ps3t")[:O, :tw]
                for k in range(MH2):
                    nc.tensor.matmul(
                        ps3,
                        w3s(k),
                        h2sb[:, k, :],
                        start=(k == 0),
                        stop=(k == MH2 - 1) and not with_bias,
                    )
                bias_mm(ps3, H + H2, H + H2 + O, tw)
                # Per-tile SWDGE (gpsimd-issued) output transfer: overlaps
                # with later tiles' compute instead of sitting on the tail,
                # and doesn't occupy a HWDGE queue.
                osb = wpool.tile([O, tw], F32, name=f"osb{t}")
                nc.vector.tensor_copy(osb, ps3)
                nc.gpsimd.dma_start(out[:, tok], osb)
    return nc


_NC_CACHE: dict = {}


def _get_nc(with_bias: bool) -> bass.Bass:
    if with_bias not in _NC_CACHE:
        _NC_CACHE[with_bias] = _build_nc(with_bias)
    return _NC_CACHE[with_bias]


def _route(x, Wr, br):
    """Host router: softmax over logits, top-2, renormalized weights."""
    logits = x.astype(np.float32) @ Wr.astype(np.float32) + br.astype(np.float32)
    m = logits.max(axis=-1, keepdims=True)
    p = np.exp(logits - m)
    p /= p.sum(axis=-1, keepdims=True)
    top_i = np.argsort(-p, axis=-1, kind="stable")[:, :TOP_K]
    top_p = np.take_along_axis(p, top_i, axis=-1)
    top_p = top_p / top_p.sum(axis=-1, keepdims=True)
    return top_i.astype(np.int64), top_p.astype(np.float32)


def _run_rounds(x, top_i, top_p, W1, b1, W2, b2, W3, b3, trace=False):
    """Dispatch tokens to expert-owning cores, run the NEFF, combine."""
    with_bias = bool(np.any(b1) or np.any(b2) or np.any(b3))
    nc = _get_nc(with_bias)

    # Static per-core weight inputs, packed into the on-chip layout:
    # [128 partitions, k-chunk-major columns] per weight matrix.
    w_maps = []
    for e in range(NCORES):
        m = {
            # w1 m-major: [p, m, k, c] so the first m-groups lead the DMA
            "w1p": np.ascontiguousarray(
                W1[e].reshape(KD, P, MH, P).transpose(1, 2, 0, 3).reshape(P, NW1)
            ).astype(_nbf16),
            "w2p": np.ascontiguousarray(
                W2[e].reshape(MH, P, H2).transpose(1, 0, 2).reshape(P, NW2)
            ).astype(_nbf16),
            "w3p": np.ascontiguousarray(
                W3[e].reshape(MH2, P, O).transpose(1, 0, 2).reshape(P, NW3)
            ).astype(_nbf16),
        }
        if with_bias:
            m["bias"] = np.concatenate(
                [b1[e], b2[e], b3[e]]
            ).reshape(1, H + H2 + O).astype(_nbf16)
        w_maps.append(m)

    # (token, slot) pairs per expert.
    tok_by_e = []
    wt_by_e = []
    for e in range(NCORES):
        tok, slot = np.nonzero(top_i == e)
        tok_by_e.append(tok)
        wt_by_e.append(top_p[tok, slot])

    out = np.zeros((B, O), np.float32)
    offset = [0] * NCORES
    last_result = None
    while True:
        active = [e for e in range(NCORES) if offset[e] < len(tok_by_e[e])]
        if not active and last_result is not None:
            break
        in_maps = []
        chunks = []
        for e in range(NCORES):
            tok = tok_by_e[e][offset[e]:offset[e] + C]
            chunks.append(tok)
            xt = np.zeros((P, KD, C), _nbf16)
            if len(tok):
                # [n, D] -> [D, n] -> k-chunks [KD, P, n] -> [P, KD, n]
                xg = x[tok].astype(_nbf16).T.reshape(KD, P, len(tok))
                xt[:, :, :len(tok)] = xg.transpose(1, 0, 2)
            in_maps.append({"xt": np.ascontiguousarray(xt), **w_maps[e]})
        res = run_bass_kernel_spmd(
            nc, in_maps, core_ids=list(range(NCORES)), trace=trace
        )
        last_result = res
        for e in range(NCORES):
            tok = chunks[e]
            if len(tok) == 0:
                continue
            y = res.results[e]["out"][:, :len(tok)].T  # [n_e, O]
            w = wt_by_e[e][offset[e]:offset[e] + len(tok)]
            np.add.at(out, tok, w[:, None] * y)
            offset[e] += len(tok)
    return out, last_result


def kernel(x, Wr, br, W1, b1, W2, b2, W3, b3):
    x = np.asarray(x, np.float32)
    top_i, top_p = _route(x, np.asarray(Wr), np.asarray(br))
    out, _ = _run_rounds(
        x, top_i, top_p,
        np.asarray(W1), np.asarray(b1), np.asarray(W2), np.asarray(b2),
        np.asarray(W3), np.asarray(b3),
    )
    return out


def run_traced(x, Wr, br, W1, b1, W2, b2, W3, b3):
    """Like kernel() but returns (out, BassKernelResults) with profile info."""
    x = np.asarray(x, np.float32)
    top_i, top_p = _route(x, np.asarray(Wr), np.asarray(br))
    return _run_rounds(
        x, top_i, top_p,
        np.asarray(W1), np.asarray(b1), np.asarray(W2), np.asarray(b2),
        np.asarray(W3), np.asarray(b3),
        trace=True,
    )


# revision 51
# speedup vs baseline: 1.3193x; 1.0447x over previous
"""MoE (top-2 of 8 experts) Trainium2 kernel.

Strategy (expert-parallel over 8 NeuronCores):
  - Router (x @ Wr -> softmax -> top-2 -> renormalize) runs on host: it is
    ~0.1% of total FLOPs and produces the token->expert dispatch that defines
    the sharding itself.
  - Each core e receives the tokens routed to expert e (gathered, transposed
    to [D, C], zero-padded to capacity C) plus expert e's weights, and runs
    the 3-layer MLP fully on-device in a transposed dataflow:
        h1T = relu(W1^T x^T + b1)   [H,  C]
        h2T = relu(W2^T h1T + b2)   [H2, C]
        yT  = W3^T h2T + b3         [O,  C]
    All matmul contractions sit on the partition axis, so no on-chip
    transposes are needed anywhere.
  - Host combines per-expert outputs with the renormalized top-2 routing
    weights (scatter-add), exactly matching the reference's dense-combine
    semantics.
  - Matmuls run in bf16 with fp32 PSUM accumulation (measured ~4e-3 max
    relative error vs the fp32 reference).
  - If any expert receives more than C tokens (never observed; capacity is
    1.25x the expected per-expert load), the leftover tokens are processed in
    an additional run of the same NEFF - correctness never depends on C.
"""

import re as _re

import numpy as np
import ml_dtypes

import bass_rust as _bass_rust
import concourse.bass as bass
import concourse.mybir as mybir
import concourse.tile as tile
from concourse.bass_utils import run_bass_kernel_spmd


def _split_drain_and_barrier(self, tick_clock, wait_clock):
    """Replacement for TileContext._drain_and_barrier.

    The stock version hangs every outstanding proc semaphore wait on one
    Drain instruction; the walrus in this environment rejects any
    instruction carrying more than one sync wait. Emit the same waits as
    individual sync-engine wait_ge instructions (one wait each) before a
    clean drain instead.
    """
    ticks = [
        int(v)
        for v in _re.findall(r"\d+", repr(tick_clock.global_clock))
    ]
    for proc, sem in sorted(self.sems.allocated().items()):
        if proc < len(ticks) and ticks[proc] > 0:
            self.nc.sync.wait_ge(sem, _bass_rust.tick_to_sem(ticks[proc], proc))
    self.nc.sync.drain()

    self.nc.all_engine_barrier()
    assert self.sems is not None
    popped = self.nc._tile_sem_poison_stack.pop()
    assert popped is self._sem_poison
    self.nc.clear_and_free_semaphores(list(self.sems.allocated().values()))
    self.nc.all_engine_barrier()


tile.TileContext._drain_and_barrier = _split_drain_and_barrier

B, D, H, E, O, TOP_K = 8192, 1024, 2048, 8, 10, 2
H2 = H // 2
NCORES = 8
P = 128

TWS = [512, 512, 512, 512, 256]   # token tile widths (<=512 = one PSUM bank)
C = sum(TWS)      # per-expert token capacity (tokens, padded)
OVERFLOW_TWS = [512]              # small NEFF for the (never-seen) case of
                                  # an expert exceeding C tokens
KD = D // P       # 8   k-chunks for layer 1
MH = H // P       # 16  m-tiles for layer 1 / k-chunks for layer 2
MH2 = H2 // P     # 8   m-tiles for layer 2 / k-chunks for layer 3

BF16 = mybir.dt.bfloat16
F32 = mybir.dt.float32
_nbf16 = ml_dtypes.bfloat16


NW1 = KD * H          # w1 columns in the packed weight tile
NW2 = MH * H2         # w2 columns
NW3 = MH2 * O         # w3 columns
NWTOT = NW1 + NW2 + NW3
W1G = 4               # w1 arrives in this many m-major group DMAs


def _build_nc(with_bias: bool, tws) -> bass.Bass:
    cap = sum(tws)
    nc = bass.Bass()
    # Host pre-packs everything into the on-chip layout:
    #  xt   [128, KD, C]  — x gathered/transposed, k-chunks on axis 1
    #  w1/w2/w3 packed k-chunk-major: [128, KD*H] etc., bf16
    xt = nc.dram_tensor("xt", [P, KD, cap], BF16, kind="ExternalInput")
    w1d = nc.dram_tensor("w1p", [P, NW1], BF16, kind="ExternalInput")
    w2d = nc.dram_tensor("w2p", [P, NW2], BF16, kind="ExternalInput")
    w3d = nc.dram_tensor("w3p", [P, NW3], BF16, kind="ExternalInput")
    if with_bias:
        # biases as single-partition rows, pre-cast to bf16 on host:
        # cols [0,H) = b1, [H,H+H2) = b2, [H+H2,H+H2+O) = b3
        bias = nc.dram_tensor("bias", [1, H + H2 + O], BF16, kind="ExternalInput")
    out = nc.dram_tensor("out", [O, cap], F32, kind="ExternalOutput")

    relu_kw = dict(op0=mybir.AluOpType.max)

    with tile.TileContext(nc) as tc:
        with (
            tc.tile_pool(name="weights", bufs=1) as wpool,
            tc.tile_pool(name="xin", bufs=1) as xpool,
            tc.tile_pool(name="ps1", bufs=2, space="PSUM") as ps1pool,
            tc.tile_pool(name="ps2", bufs=2, space="PSUM") as ps2pool,
            tc.tile_pool(name="ps3", bufs=2, space="PSUM") as ps3pool,
            tc.tile_pool(name="acts", bufs=2) as apool,
        ):
            # w1 and the first x tile gate the first matmul group — give
            # each its own DMA (own HWDGE queue) so they stream in
            # parallel instead of behind one monolithic transfer. w1 is
            # further packed m-major and split into W1G group-DMAs with
            # separate tiles, so the m-loop's first groups start while the
            # rest of w1 is still in flight.
            MG = MH // W1G        # m-tiles per w1 group
            w1g_tiles = []
            for g in range(W1G):
                w1g = wpool.tile([P, MG * KD * P], BF16, name=f"w1g{g}")
                nc.sync.dma_start(
                    w1g, w1d[:, g * MG * KD * P:(g + 1) * MG * KD * P])
                w1g_tiles.append(w1g)
            xsb_tiles = []
            off = 0
            for t, tw in enumerate(tws):
                xsb = xpool.tile([P, KD, tw], BF16, tag=f"x{t}")
                nc.sync.dma_start(xsb, xt[:, :, off:off + tw])
                xsb_tiles.append(xsb)
                off += tw
            w2sb = wpool.tile([P, NW2], BF16)
            nc.sync.dma_start(w2sb, w2d[:, :])
            w3sb = wpool.tile([P, NW3], BF16)
            nc.sync.dma_start(w3sb, w3d[:, :])

            def w1s(k, m):
                g, mm_ = divmod(m, MG)
                off = (mm_ * KD + k) * P
                return w1g_tiles[g][:, off:off + P]

            def w2s(k, m):
                off = k * H2 + m * P
                return w2sb[:, off:off + P]

            def w3s(k):
                off = k * O
                return w3sb[:, off:off + O]

            if with_bias:
                # Bias folded into each accumulation group as one extra K=1
                # matmul against a ones row: psum[m, n] += b[m] * 1. This
                # keeps bias handling entirely on the PE, so no evacuation
                # instruction ever needs a second semaphore wait.
                bsb = wpool.tile([1, H + H2 + O], BF16)
                nc.sync.dma_start(bsb, bias[:, :])
                ones = wpool.tile([1, max(tws)], BF16)
                nc.vector.memset(ones, 1.0)

            def bias_mm(ps, lo, hi, tw):
                if with_bias:
                    nc.tensor.matmul(
                        ps, bsb[:, lo:hi], ones[:, :tw], start=False, stop=True
                    )

            # 1-element DVE reads of the previous tile's activation buffers.
            # Slot reuse makes the first evacuation of a tile WAW-depend on
            # the previous tile's writes; the fence absorbs that own-engine
            # completion wait so no evacuation needs two semaphore waits
            # (the ISA wait slot fits only one).
            fence = wpool.tile([1, 4], BF16)
            prev = {}

            def dve_fence(key, ap):
                if key in prev:
                    nc.vector.tensor_copy(fence[:, 0:1], prev[key])
                prev[key] = ap



            tok_off = 0
            for t, tw in enumerate(tws):
                tok = slice(tok_off, tok_off + tw)
                tok_off += tw
                xsb = xsb_tiles[t]

                h1sb = apool.tile([P, MH, tw], BF16, tag="h1")
                dve_fence("h1", h1sb[0:1, 0, 0:1])
                for m in range(MH):
                    ps = ps1pool.tile([P, 512], F32, tag="ps1", name="ps1t")[:, :tw]
                    for k in range(KD):
                        nc.tensor.matmul(
                            ps,
                            w1s(k, m),
                            xsb[:, k, :],
                            start=(k == 0),
                            stop=(k == KD - 1) and not with_bias,
                        )
                    bias_mm(ps, m * P, (m + 1) * P, tw)
                    nc.vector.tensor_scalar(
                        h1sb[:, m, :], ps, 0.0, None, **relu_kw
                    )

                h2sb = apool.tile([P, MH2, tw], BF16, tag="h2")
                dve_fence("h2", h2sb[0:1, 0, 0:1])
                for m in range(MH2):
                    ps = ps2pool.tile([P, 512], F32, tag="ps2", name="ps2t")[:, :tw]
                    for k in range(MH):
                        nc.tensor.matmul(
                            ps,
                            w2s(k, m),
                            h1sb[:, k, :],
                            start=(k == 0),
                            stop=(k == MH - 1) and not with_bias,
                        )
                    bias_mm(ps, H + m * P, H + (m + 1) * P, tw)
                    nc.vector.tensor_scalar(
                        h2sb[:, m, :], ps, 0.0, None, **relu_kw
                    )

                ps3 = ps3pool.tile([P, 512], F32, tag="ps3", name="ps3t")[:O, :tw]
                for k in range(MH2):
                    nc.tensor.matmul(
                        ps3,
                        w3s(k),
                        h2sb[:, k, :],
                        start=(k == 0),
                        stop=(k == MH2 - 1) and not with_bias,
                    )
                bias_mm(ps3, H + H2, H + H2 + O, tw)
                # Per-tile SWDGE (gpsimd-issued) output transfer: overlaps
                # with later tiles' compute instead of sitting on the tail,
                # and doesn't occupy a HWDGE queue.
                osb = wpool.tile([O, tw], F32, name=f"osb{t}")
                nc.vector.tensor_copy(osb, ps3)
                nc.gpsimd.dma_start(out[:, tok], osb)
    return nc


_NC_CACHE: dict = {}


def _get_nc(with_bias: bool, tws) -> bass.Bass:
    key = (with_bias, tuple(tws))
    if key not in _NC_CACHE:
        _NC_CACHE[key] = _build_nc(with_bias, tws)
    return _NC_CACHE[key]


def _route(x, Wr, br):
    """Host router: softmax over logits, top-2, renormalized weights."""
    logits = x.astype(np.float32) @ Wr.astype(np.float32) + br.astype(np.float32)
    m = logits.max(axis=-1, keepdims=True)
    p = np.exp(logits - m)
    p /= p.sum(axis=-1, keepdims=True)
    top_i = np.argsort(-p, axis=-1, kind="stable")[:, :TOP_K]
    top_p = np.take_along_axis(p, top_i, axis=-1)
    top_p = top_p / top_p.sum(axis=-1, keepdims=True)
    return top_i.astype(np.int64), top_p.astype(np.float32)


def _run_rounds(x, top_i, top_p, W1, b1, W2, b2, W3, b3, trace=False):
    """Dispatch tokens to expert-owning cores, run the NEFF, combine."""
    with_bias = bool(np.any(b1) or np.any(b2) or np.any(b3))

    # Static per-core weight inputs, packed into the on-chip layout:
    # [128 partitions, k-chunk-major columns] per weight matrix.
    w_maps = []
    for e in range(NCORES):
        m = {
            # w1 m-major: [p, m, k, c] so the first m-groups lead the DMA
            "w1p": np.ascontiguousarray(
                W1[e].reshape(KD, P, MH, P).transpose(1, 2, 0, 3).reshape(P, NW1)
            ).astype(_nbf16),
            "w2p": np.ascontiguousarray(
                W2[e].reshape(MH, P, H2).transpose(1, 0, 2).reshape(P, NW2)
            ).astype(_nbf16),
            "w3p": np.ascontiguousarray(
                W3[e].reshape(MH2, P, O).transpose(1, 0, 2).reshape(P, NW3)
            ).astype(_nbf16),
        }
        if with_bias:
            m["bias"] = np.concatenate(
                [b1[e], b2[e], b3[e]]
            ).reshape(1, H + H2 + O).astype(_nbf16)
        w_maps.append(m)

    # (token, slot) pairs per expert.
    tok_by_e = []
    wt_by_e = []
    for e in range(NCORES):
        tok, slot = np.nonzero(top_i == e)
        tok_by_e.append(tok)
        wt_by_e.append(top_p[tok, slot])

    out = np.zeros((B, O), np.float32)
    offset = [0] * NCORES
    last_result = None
    first_round = True
    while True:
        active = [e for e in range(NCORES) if offset[e] < len(tok_by_e[e])]
        if not active and last_result is not None:
            break
        # Round 1 uses the full-capacity NEFF. In the (never-observed) case
        # that an expert got more than C tokens, the leftovers run through a
        # small single-tile NEFF instead of paying for a full rerun.
        tws = TWS if first_round else OVERFLOW_TWS
        cap = sum(tws)
        nc = _get_nc(with_bias, tws)
        first_round = False
        in_maps = []
        chunks = []
        for e in range(NCORES):
            tok = tok_by_e[e][offset[e]:offset[e] + cap]
            chunks.append(tok)
            xt = np.zeros((P, KD, cap), _nbf16)
            if len(tok):
                # [n, D] -> [D, n] -> k-chunks [KD, P, n] -> [P, KD, n]
                xg = x[tok].astype(_nbf16).T.reshape(KD, P, len(tok))
                xt[:, :, :len(tok)] = xg.transpose(1, 0, 2)
            in_maps.append({"xt": np.ascontiguousarray(xt), **w_maps[e]})
        res = run_bass_kernel_spmd(
            nc, in_maps, core_ids=list(range(NCORES)), trace=trace
        )
        last_result = res
        for e in range(NCORES):
            tok = chunks[e]
            if len(tok) == 0:
                continue
            y = res.results[e]["out"][:, :len(tok)].T  # [n_e, O]
            w = wt_by_e[e][offset[e]:offset[e] + len(tok)]
            np.add.at(out, tok, w[:, None] * y)
            offset[e] += len(tok)
    return out, last_result


def kernel(x, Wr, br, W1, b1, W2, b2, W3, b3):
    x = np.asarray(x, np.float32)
    top_i, top_p = _route(x, np.asarray(Wr), np.asarray(br))
    out, _ = _run_rounds(
        x, top_i, top_p,
        np.asarray(W1), np.asarray(b1), np.asarray(W2), np.asarray(b2),
        np.asarray(W3), np.asarray(b3),
    )
    return out


def run_traced(x, Wr, br, W1, b1, W2, b2, W3, b3):
    """Like kernel() but returns (out, BassKernelResults) with profile info."""
    x = np.asarray(x, np.float32)
    top_i, top_p = _route(x, np.asarray(Wr), np.asarray(br))
    return _run_rounds(
        x, top_i, top_p,
        np.asarray(W1), np.asarray(b1), np.asarray(W2), np.asarray(b2),
        np.asarray(W3), np.asarray(b3),
        trace=True,
    )


# revision 53
# speedup vs baseline: 1.3264x; 1.0054x over previous
"""MoE (top-2 of 8 experts) Trainium2 kernel.

Strategy (expert-parallel over 8 NeuronCores):
  - Router (x @ Wr -> softmax -> top-2 -> renormalize) runs on host: it is
    ~0.1% of total FLOPs and produces the token->expert dispatch that defines
    the sharding itself.
  - Each core e receives the tokens routed to expert e (gathered, transposed
    to [D, C], zero-padded to capacity C) plus expert e's weights, and runs
    the 3-layer MLP fully on-device in a transposed dataflow:
        h1T = relu(W1^T x^T + b1)   [H,  C]
        h2T = relu(W2^T h1T + b2)   [H2, C]
        yT  = W3^T h2T + b3         [O,  C]
    All matmul contractions sit on the partition axis, so no on-chip
    transposes are needed anywhere.
  - Host combines per-expert outputs with the renormalized top-2 routing
    weights (scatter-add), exactly matching the reference's dense-combine
    semantics.
  - Matmuls run in bf16 with fp32 PSUM accumulation (measured ~4e-3 max
    relative error vs the fp32 reference).
  - If any expert receives more than C tokens (never observed; capacity is
    1.25x the expected per-expert load), the leftover tokens are processed in
    an additional run of the same NEFF - correctness never depends on C.
"""

import re as _re

import numpy as np
import ml_dtypes

import bass_rust as _bass_rust
import concourse.bass as bass
import concourse.mybir as mybir
import concourse.tile as tile
from concourse.bass_utils import run_bass_kernel_spmd


def _split_drain_and_barrier(self, tick_clock, wait_clock):
    """Replacement for TileContext._drain_and_barrier.

    The stock version hangs every outstanding proc semaphore wait on one
    Drain instruction; the walrus in this environment rejects any
    instruction carrying more than one sync wait. Emit the same waits as
    individual sync-engine wait_ge instructions (one wait each) before a
    clean drain instead.
    """
    ticks = [
        int(v)
        for v in _re.findall(r"\d+", repr(tick_clock.global_clock))
    ]
    for proc, sem in sorted(self.sems.allocated().items()):
        if proc < len(ticks) and ticks[proc] > 0:
            self.nc.sync.wait_ge(sem, _bass_rust.tick_to_sem(ticks[proc], proc))
    self.nc.sync.drain()

    self.nc.all_engine_barrier()
    assert self.sems is not None
    popped = self.nc._tile_sem_poison_stack.pop()
    assert popped is self._sem_poison
    self.nc.clear_and_free_semaphores(list(self.sems.allocated().values()))
    self.nc.all_engine_barrier()


tile.TileContext._drain_and_barrier = _split_drain_and_barrier

B, D, H, E, O, TOP_K = 8192, 1024, 2048, 8, 10, 2
H2 = H // 2
NCORES = 8
P = 128

TWS = [512, 512, 512, 512, 256]   # token tile widths (<=512 = one PSUM bank)
C = sum(TWS)      # per-expert token capacity (tokens, padded)
OVERFLOW_TWS = [512]              # small NEFF for the (never-seen) case of
                                  # an expert exceeding C tokens
KD = D // P       # 8   k-chunks for layer 1
MH = H // P       # 16  m-tiles for layer 1 / k-chunks for layer 2
MH2 = H2 // P     # 8   m-tiles for layer 2 / k-chunks for layer 3

BF16 = mybir.dt.bfloat16
F32 = mybir.dt.float32
_nbf16 = ml_dtypes.bfloat16


NW1 = KD * H          # w1 columns in the packed weight tile
NW2 = MH * H2         # w2 columns
NW3 = MH2 * O         # w3 columns
NWTOT = NW1 + NW2 + NW3
W1G = 4               # w1 arrives in this many m-major group DMAs


def _build_nc(with_bias: bool, tws) -> bass.Bass:
    cap = sum(tws)
    nc = bass.Bass()
    # Host pre-packs everything into the on-chip layout:
    #  xt   [128, KD, C]  — x gathered/transposed, k-chunks on axis 1
    #  w1/w2/w3 packed k-chunk-major: [128, KD*H] etc., bf16
    xt = nc.dram_tensor("xt", [P, KD, cap], BF16, kind="ExternalInput")
    w1d = nc.dram_tensor("w1p", [P, NW1], BF16, kind="ExternalInput")
    w2d = nc.dram_tensor("w2p", [P, NW2], BF16, kind="ExternalInput")
    w3d = nc.dram_tensor("w3p", [P, NW3], BF16, kind="ExternalInput")
    if with_bias:
        # biases as single-partition rows, pre-cast to bf16 on host:
        # cols [0,H) = b1, [H,H+H2) = b2, [H+H2,H+H2+O) = b3
        bias = nc.dram_tensor("bias", [1, H + H2 + O], BF16, kind="ExternalInput")
    out = nc.dram_tensor("out", [O, cap], F32, kind="ExternalOutput")

    relu_kw = dict(op0=mybir.AluOpType.max)

    with tile.TileContext(nc) as tc:
        with (
            tc.tile_pool(name="weights", bufs=1) as wpool,
            tc.tile_pool(name="xin", bufs=1) as xpool,
            tc.tile_pool(name="ps1", bufs=2, space="PSUM") as ps1pool,
            tc.tile_pool(name="ps2", bufs=2, space="PSUM") as ps2pool,
            tc.tile_pool(name="ps3", bufs=2, space="PSUM") as ps3pool,
            tc.tile_pool(name="acts", bufs=2) as apool,
        ):
            # w1 and the first x tile gate the first matmul group — give
            # each its own DMA (own HWDGE queue) so they stream in
            # parallel instead of behind one monolithic transfer. w1 is
            # further packed m-major and split into W1G group-DMAs with
            # separate tiles, so the m-loop's first groups start while the
            # rest of w1 is still in flight.
            MG = MH // W1G        # m-tiles per w1 group
            w1g_tiles = []
            last_w1_dma = None
            for g in range(W1G):
                w1g = wpool.tile([P, MG * KD * P], BF16, name=f"w1g{g}")
                last_w1_dma = nc.sync.dma_start(
                    w1g, w1d[:, g * MG * KD * P:(g + 1) * MG * KD * P])
                w1g_tiles.append(w1g)
            xsb_tiles = []
            off = 0
            for t, tw in enumerate(tws):
                xsb = xpool.tile([P, KD, tw], BF16, tag=f"x{t}")
                nc.sync.dma_start(xsb, xt[:, :, off:off + tw])
                xsb_tiles.append(xsb)
                off += tw
            w2sb = wpool.tile([P, NW2], BF16)
            nc.sync.dma_start(w2sb, w2d[:, :])
            w3sb = wpool.tile([P, NW3], BF16)
            nc.sync.dma_start(w3sb, w3d[:, :])

            def w1s(k, m):
                g, mm_ = divmod(m, MG)
                off = (mm_ * KD + k) * P
                return w1g_tiles[g][:, off:off + P]

            def w2s(k, m):
                off = k * H2 + m * P
                return w2sb[:, off:off + P]

            def w3s(k):
                off = k * O
                return w3sb[:, off:off + O]

            if with_bias:
                # Bias folded into each accumulation group as one extra K=1
                # matmul against a ones row: psum[m, n] += b[m] * 1. This
                # keeps bias handling entirely on the PE, so no evacuation
                # instruction ever needs a second semaphore wait.
                bsb = wpool.tile([1, H + H2 + O], BF16)
                nc.sync.dma_start(bsb, bias[:, :])
                ones = wpool.tile([1, max(tws)], BF16)
                nc.vector.memset(ones, 1.0)

            def bias_mm(ps, lo, hi, tw):
                if with_bias:
                    nc.tensor.matmul(
                        ps, bsb[:, lo:hi], ones[:, :tw], start=False, stop=True
                    )

            # 1-element DVE reads of the previous tile's activation buffers.
            # Slot reuse makes the first evacuation of a tile WAW-depend on
            # the previous tile's writes; the fence absorbs that own-engine
            # completion wait so no evacuation needs two semaphore waits
            # (the ISA wait slot fits only one).
            fence = wpool.tile([1, 4], BF16)
            prev = {}

            def dve_fence(key, ap):
                if key in prev:
                    nc.vector.tensor_copy(fence[:, 0:1], prev[key])
                prev[key] = ap



            tok_off = 0
            for t, tw in enumerate(tws):
                tok = slice(tok_off, tok_off + tw)
                tok_off += tw
                xsb = xsb_tiles[t]

                h1sb = apool.tile([P, MH, tw], BF16, tag="h1")
                dve_fence("h1", h1sb[0:1, 0, 0:1])
                for m in range(MH):
                    ps = ps1pool.tile([P, 512], F32, tag="ps1", name="ps1t")[:, :tw]
                    for k in range(KD):
                        nc.tensor.matmul(
                            ps,
                            w1s(k, m),
                            xsb[:, k, :],
                            start=(k == 0),
                            stop=(k == KD - 1) and not with_bias,
                        )
                    bias_mm(ps, m * P, (m + 1) * P, tw)
                    nc.vector.tensor_scalar(
                        h1sb[:, m, :], ps, 0.0, None, **relu_kw
                    )

                h2sb = apool.tile([P, MH2, tw], BF16, tag="h2")
                dve_fence("h2", h2sb[0:1, 0, 0:1])
                for m in range(MH2):
                    ps = ps2pool.tile([P, 512], F32, tag="ps2", name="ps2t")[:, :tw]
                    for k in range(MH):
                        nc.tensor.matmul(
                            ps,
                            w2s(k, m),
                            h1sb[:, k, :],
                            start=(k == 0),
                            stop=(k == MH - 1) and not with_bias,
                        )
                    bias_mm(ps, H + m * P, H + (m + 1) * P, tw)
                    nc.vector.tensor_scalar(
                        h2sb[:, m, :], ps, 0.0, None, **relu_kw
                    )

                ps3 = ps3pool.tile([P, 512], F32, tag="ps3", name="ps3t")[:O, :tw]
                for k in range(MH2):
                    nc.tensor.matmul(
                        ps3,
                        w3s(k),
                        h2sb[:, k, :],
                        start=(k == 0),
                        stop=(k == MH2 - 1) and not with_bias,
                    )
                bias_mm(ps3, H + H2, H + H2 + O, tw)
                # Per-tile SWDGE (gpsimd-issued) output transfer: overlaps
                # with later tiles' compute instead of sitting on the tail,
                # and doesn't occupy a HWDGE queue.
                osb = wpool.tile([O, tw], F32, name=f"osb{t}")
                nc.vector.tensor_copy(osb, ps3)
                nc.gpsimd.dma_start(out[:, tok], osb)
    return nc


_NC_CACHE: dict = {}


def _get_nc(with_bias: bool, tws) -> bass.Bass:
    key = (with_bias, tuple(tws))
    if key not in _NC_CACHE:
        _NC_CACHE[key] = _build_nc(with_bias, tws)
    return _NC_CACHE[key]


def _route(x, Wr, br):
    """Host router: softmax over logits, top-2, renormalized weights."""
    logits = x.astype(np.float32) @ Wr.astype(np.float32) + br.astype(np.float32)
    m = logits.max(axis=-1, keepdims=True)
    p = np.exp(logits - m)
    p /= p.sum(axis=-1, keepdims=True)
    top_i = np.argsort(-p, axis=-1, kind="stable")[:, :TOP_K]
    top_p = np.take_along_axis(p, top_i, axis=-1)
    top_p = top_p / top_p.sum(axis=-1, keepdims=True)
    return top_i.astype(np.int64), top_p.astype(np.float32)


def _run_rounds(x, top_i, top_p, W1, b1, W2, b2, W3, b3, trace=False):
    """Dispatch tokens to expert-owning cores, run the NEFF, combine."""
    with_bias = bool(np.any(b1) or np.any(b2) or np.any(b3))

    # Static per-core weight inputs, packed into the on-chip layout:
    # [128 partitions, k-chunk-major columns] per weight matrix.
    w_maps = []
    for e in range(NCORES):
        m = {
            # w1 m-major: [p, m, k, c] so the first m-groups lead the DMA
            "w1p": np.ascontiguousarray(
                W1[e].reshape(KD, P, MH, P).transpose(1, 2, 0, 3).reshape(P, NW1)
            ).astype(_nbf16),
            "w2p": np.ascontiguousarray(
                W2[e].reshape(MH, P, H2).transpose(1, 0, 2).reshape(P, NW2)
            ).astype(_nbf16),
            "w3p": np.ascontiguousarray(
                W3[e].reshape(MH2, P, O).transpose(1, 0, 2).reshape(P, NW3)
            ).astype(_nbf16),
        }
        if with_bias:
            m["bias"] = np.concatenate(
                [b1[e], b2[e], b3[e]]
            ).reshape(1, H + H2 + O).astype(_nbf16)
        w_maps.append(m)

    # (token, slot) pairs per expert.
    tok_by_e = []
    wt_by_e = []
    for e in range(NCORES):
        tok, slot = np.nonzero(top_i == e)
        tok_by_e.append(tok)
        wt_by_e.append(top_p[tok, slot])

    out = np.zeros((B, O), np.float32)
    offset = [0] * NCORES
    last_result = None
    first_round = True
    while True:
        active = [e for e in range(NCORES) if offset[e] < len(tok_by_e[e])]
        if not active and last_result is not None:
            break
        # Round 1 uses the full-capacity NEFF. In the (never-observed) case
        # that an expert got more than C tokens, the leftovers run through a
        # small single-tile NEFF instead of paying for a full rerun.
        tws = TWS if first_round else OVERFLOW_TWS
        cap = sum(tws)
        nc = _get_nc(with_bias, tws)
        first_round = False
        in_maps = []
        chunks = []
        for e in range(NCORES):
            tok = tok_by_e[e][offset[e]:offset[e] + cap]
            chunks.append(tok)
            xt = np.zeros((P, KD, cap), _nbf16)
            if len(tok):
                # [n, D] -> [D, n] -> k-chunks [KD, P, n] -> [P, KD, n]
                xg = x[tok].astype(_nbf16).T.reshape(KD, P, len(tok))
                xt[:, :, :len(tok)] = xg.transpose(1, 0, 2)
            in_maps.append({"xt": np.ascontiguousarray(xt), **w_maps[e]})
        res = run_bass_kernel_spmd(
            nc, in_maps, core_ids=list(range(NCORES)), trace=trace
        )
        last_result = res
        for e in range(NCORES):
            tok = chunks[e]
            if len(tok) == 0:
                continue
            y = res.results[e]["out"][:, :len(tok)].T  # [n_e, O]
            w = wt_by_e[e][offset[e]:offset[e] + len(tok)]
            np.add.at(out, tok, w[:, None] * y)
            offset[e] += len(tok)
    return out, last_result


def kernel(x, Wr, br, W1, b1, W2, b2, W3, b3):
    x = np.asarray(x, np.float32)
    top_i, top_p = _route(x, np.asarray(Wr), np.asarray(br))
    out, _ = _run_rounds(
        x, top_i, top_p,
        np.asarray(W1), np.asarray(b1), np.asarray(W2), np.asarray(b2),
        np.asarray(W3), np.asarray(b3),
    )
    return out


def run_traced(x, Wr, br, W1, b1, W2, b2, W3, b3):
    """Like kernel() but returns (out, BassKernelResults) with profile info."""
    x = np.asarray(x, np.float32)
    top_i, top_p = _route(x, np.asarray(Wr), np.asarray(br))
    return _run_rounds(
        x, top_i, top_p,
        np.asarray(W1), np.asarray(b1), np.asarray(W2), np.asarray(b2),
        np.asarray(W3), np.asarray(b3),
        trace=True,
    )


# revision 54
# speedup vs baseline: 1.3444x; 1.0136x over previous
"""MoE (top-2 of 8 experts) Trainium2 kernel.

Strategy (expert-parallel over 8 NeuronCores):
  - Router (x @ Wr -> softmax -> top-2 -> renormalize) runs on host: it is
    ~0.1% of total FLOPs and produces the token->expert dispatch that defines
    the sharding itself.
  - Each core e receives the tokens routed to expert e (gathered, transposed
    to [D, C], zero-padded to capacity C) plus expert e's weights, and runs
    the 3-layer MLP fully on-device in a transposed dataflow:
        h1T = relu(W1^T x^T + b1)   [H,  C]
        h2T = relu(W2^T h1T + b2)   [H2, C]
        yT  = W3^T h2T + b3         [O,  C]
    All matmul contractions sit on the partition axis, so no on-chip
    transposes are needed anywhere.
  - Host combines per-expert outputs with the renormalized top-2 routing
    weights (scatter-add), exactly matching the reference's dense-combine
    semantics.
  - Matmuls run in bf16 with fp32 PSUM accumulation (measured ~4e-3 max
    relative error vs the fp32 reference).
  - If any expert receives more than C tokens (never observed; capacity is
    1.25x the expected per-expert load), the leftover tokens are processed in
    an additional run of the same NEFF - correctness never depends on C.
"""

import re as _re

import numpy as np
import ml_dtypes

import bass_rust as _bass_rust
import concourse.bass as bass
import concourse.mybir as mybir
import concourse.tile as tile
from concourse.bass_utils import run_bass_kernel_spmd


def _split_drain_and_barrier(self, tick_clock, wait_clock):
    """Replacement for TileContext._drain_and_barrier.

    The stock version hangs every outstanding proc semaphore wait on one
    Drain instruction; the walrus in this environment rejects any
    instruction carrying more than one sync wait. Emit the same waits as
    individual sync-engine wait_ge instructions (one wait each) before a
    clean drain instead.
    """
    ticks = [
        int(v)
        for v in _re.findall(r"\d+", repr(tick_clock.global_clock))
    ]
    for proc, sem in sorted(self.sems.allocated().items()):
        if proc < len(ticks) and ticks[proc] > 0:
            self.nc.sync.wait_ge(sem, _bass_rust.tick_to_sem(ticks[proc], proc))
    self.nc.sync.drain()

    self.nc.all_engine_barrier()
    assert self.sems is not None
    popped = self.nc._tile_sem_poison_stack.pop()
    assert popped is self._sem_poison
    self.nc.clear_and_free_semaphores(list(self.sems.allocated().values()))
    self.nc.all_engine_barrier()


tile.TileContext._drain_and_barrier = _split_drain_and_barrier

B, D, H, E, O, TOP_K = 8192, 1024, 2048, 8, 10, 2
H2 = H // 2
NCORES = 8
P = 128

TWS = [512, 512, 512, 512, 192]   # token tile widths (<=512 = one PSUM bank)
C = sum(TWS)      # per-expert token capacity (tokens, padded)
OVERFLOW_TWS = [512]              # small NEFF for the (never-seen) case of
                                  # an expert exceeding C tokens
KD = D // P       # 8   k-chunks for layer 1
MH = H // P       # 16  m-tiles for layer 1 / k-chunks for layer 2
MH2 = H2 // P     # 8   m-tiles for layer 2 / k-chunks for layer 3

BF16 = mybir.dt.bfloat16
F32 = mybir.dt.float32
_nbf16 = ml_dtypes.bfloat16


NW1 = KD * H          # w1 columns in the packed weight tile
NW2 = MH * H2         # w2 columns
NW3 = MH2 * O         # w3 columns
NWTOT = NW1 + NW2 + NW3
W1G = 4               # w1 arrives in this many m-major group DMAs


def _build_nc(with_bias: bool, tws) -> bass.Bass:
    cap = sum(tws)
    nc = bass.Bass()
    # Host pre-packs everything into the on-chip layout:
    #  xt   [128, KD, C]  — x gathered/transposed, k-chunks on axis 1
    #  w1/w2/w3 packed k-chunk-major: [128, KD*H] etc., bf16
    xt = nc.dram_tensor("xt", [P, KD, cap], BF16, kind="ExternalInput")
    w1d = nc.dram_tensor("w1p", [P, NW1], BF16, kind="ExternalInput")
    w2d = nc.dram_tensor("w2p", [P, NW2], BF16, kind="ExternalInput")
    w3d = nc.dram_tensor("w3p", [P, NW3], BF16, kind="ExternalInput")
    if with_bias:
        # biases as single-partition rows, pre-cast to bf16 on host:
        # cols [0,H) = b1, [H,H+H2) = b2, [H+H2,H+H2+O) = b3
        bias = nc.dram_tensor("bias", [1, H + H2 + O], BF16, kind="ExternalInput")
    out = nc.dram_tensor("out", [O, cap], F32, kind="ExternalOutput")

    relu_kw = dict(op0=mybir.AluOpType.max)

    with tile.TileContext(nc) as tc:
        with (
            tc.tile_pool(name="weights", bufs=1) as wpool,
            tc.tile_pool(name="xin", bufs=1) as xpool,
            tc.tile_pool(name="ps1", bufs=2, space="PSUM") as ps1pool,
            tc.tile_pool(name="ps2", bufs=2, space="PSUM") as ps2pool,
            tc.tile_pool(name="ps3", bufs=2, space="PSUM") as ps3pool,
            tc.tile_pool(name="acts", bufs=2) as apool,
        ):
            # w1 and the first x tile gate the first matmul group — give
            # each its own DMA (own HWDGE queue) so they stream in
            # parallel instead of behind one monolithic transfer. w1 is
            # further packed m-major and split into W1G group-DMAs with
            # separate tiles, so the m-loop's first groups start while the
            # rest of w1 is still in flight.
            MG = MH // W1G        # m-tiles per w1 group
            w1g_tiles = []
            last_w1_dma = None
            for g in range(W1G):
                w1g = wpool.tile([P, MG * KD * P], BF16, name=f"w1g{g}")
                last_w1_dma = nc.sync.dma_start(
                    w1g, w1d[:, g * MG * KD * P:(g + 1) * MG * KD * P])
                w1g_tiles.append(w1g)
            xsb_tiles = []
            off = 0
            for t, tw in enumerate(tws):
                xsb = xpool.tile([P, KD, tw], BF16, tag=f"x{t}")
                nc.sync.dma_start(xsb, xt[:, :, off:off + tw])
                xsb_tiles.append(xsb)
                off += tw
            w2sb = wpool.tile([P, NW2], BF16)
            nc.sync.dma_start(w2sb, w2d[:, :])
            w3sb = wpool.tile([P, NW3], BF16)
            nc.sync.dma_start(w3sb, w3d[:, :])

            def w1s(k, m):
                g, mm_ = divmod(m, MG)
                off = (mm_ * KD + k) * P
                return w1g_tiles[g][:, off:off + P]

            def w2s(k, m):
                off = k * H2 + m * P
                return w2sb[:, off:off + P]

            def w3s(k):
                off = k * O
                return w3sb[:, off:off + O]

            if with_bias:
                # Bias folded into each accumulation group as one extra K=1
                # matmul against a ones row: psum[m, n] += b[m] * 1. This
                # keeps bias handling entirely on the PE, so no evacuation
                # instruction ever needs a second semaphore wait.
                bsb = wpool.tile([1, H + H2 + O], BF16)
                nc.sync.dma_start(bsb, bias[:, :])
                ones = wpool.tile([1, max(tws)], BF16)
                nc.vector.memset(ones, 1.0)

            def bias_mm(ps, lo, hi, tw):
                if with_bias:
                    nc.tensor.matmul(
                        ps, bsb[:, lo:hi], ones[:, :tw], start=False, stop=True
                    )

            # 1-element DVE reads of the previous tile's activation buffers.
            # Slot reuse makes the first evacuation of a tile WAW-depend on
            # the previous tile's writes; the fence absorbs that own-engine
            # completion wait so no evacuation needs two semaphore waits
            # (the ISA wait slot fits only one).
            fence = wpool.tile([1, 4], BF16)
            prev = {}

            def dve_fence(key, ap):
                if key in prev:
                    nc.vector.tensor_copy(fence[:, 0:1], prev[key])
                prev[key] = ap



            tok_off = 0
            for t, tw in enumerate(tws):
                tok = slice(tok_off, tok_off + tw)
                tok_off += tw
                xsb = xsb_tiles[t]

                h1sb = apool.tile([P, MH, tw], BF16, tag="h1")
                dve_fence("h1", h1sb[0:1, 0, 0:1])
                for m in range(MH):
                    ps = ps1pool.tile([P, 512], F32, tag="ps1", name="ps1t")[:, :tw]
                    for k in range(KD):
                        nc.tensor.matmul(
                            ps,
                            w1s(k, m),
                            xsb[:, k, :],
                            start=(k == 0),
                            stop=(k == KD - 1) and not with_bias,
                        )
                    bias_mm(ps, m * P, (m + 1) * P, tw)
                    nc.vector.tensor_scalar(
                        h1sb[:, m, :], ps, 0.0, None, **relu_kw
                    )

                h2sb = apool.tile([P, MH2, tw], BF16, tag="h2")
                dve_fence("h2", h2sb[0:1, 0, 0:1])
                for m in range(MH2):
                    ps = ps2pool.tile([P, 512], F32, tag="ps2", name="ps2t")[:, :tw]
                    for k in range(MH):
                        nc.tensor.matmul(
                            ps,
                            w2s(k, m),
                            h1sb[:, k, :],
                            start=(k == 0),
                            stop=(k == MH - 1) and not with_bias,
                        )
                    bias_mm(ps, H + m * P, H + (m + 1) * P, tw)
                    nc.vector.tensor_scalar(
                        h2sb[:, m, :], ps, 0.0, None, **relu_kw
                    )

                ps3 = ps3pool.tile([P, 512], F32, tag="ps3", name="ps3t")[:O, :tw]
                for k in range(MH2):
                    nc.tensor.matmul(
                        ps3,
                        w3s(k),
                        h2sb[:, k, :],
                        start=(k == 0),
                        stop=(k == MH2 - 1) and not with_bias,
                    )
                bias_mm(ps3, H + H2, H + H2 + O, tw)
                # Per-tile SWDGE (gpsimd-issued) output transfer: overlaps
                # with later tiles' compute instead of sitting on the tail,
                # and doesn't occupy a HWDGE queue.
                osb = wpool.tile([O, tw], F32, name=f"osb{t}")
                nc.vector.tensor_copy(osb, ps3)
                nc.gpsimd.dma_start(out[:, tok], osb)
    return nc


_NC_CACHE: dict = {}


def _get_nc(with_bias: bool, tws) -> bass.Bass:
    key = (with_bias, tuple(tws))
    if key not in _NC_CACHE:
        _NC_CACHE[key] = _build_nc(with_bias, tws)
    return _NC_CACHE[key]


def _route(x, Wr, br):
    """Host router: softmax over logits, top-2, renormalized weights."""
    logits = x.astype(np.float32) @ Wr.astype(np.float32) + br.astype(np.float32)
    m = logits.max(axis=-1, keepdims=True)
    p = np.exp(logits - m)
    p /= p.sum(axis=-1, keepdims=True)
    top_i = np.argsort(-p, axis=-1, kind="stable")[:, :TOP_K]
    top_p = np.take_along_axis(p, top_i, axis=-1)
    top_p = top_p / top_p.sum(axis=-1, keepdims=True)
    return top_i.astype(np.int64), top_p.astype(np.float32)


def _run_rounds(x, top_i, top_p, W1, b1, W2, b2, W3, b3, trace=False):
    """Dispatch tokens to expert-owning cores, run the NEFF, combine."""
    with_bias = bool(np.any(b1) or np.any(b2) or np.any(b3))

    # Static per-core weight inputs, packed into the on-chip layout:
    # [128 partitions, k-chunk-major columns] per weight matrix.
    w_maps = []
    for e in range(NCORES):
        m = {
            # w1 m-major: [p, m, k, c] so the first m-groups lead the DMA
            "w1p": np.ascontiguousarray(
                W1[e].reshape(KD, P, MH, P).transpose(1, 2, 0, 3).reshape(P, NW1)
            ).astype(_nbf16),
            "w2p": np.ascontiguousarray(
                W2[e].reshape(MH, P, H2).transpose(1, 0, 2).reshape(P, NW2)
            ).astype(_nbf16),
            "w3p": np.ascontiguousarray(
                W3[e].reshape(MH2, P, O).transpose(1, 0, 2).reshape(P, NW3)
            ).astype(_nbf16),
        }
        if with_bias:
            m["bias"] = np.concatenate(
                [b1[e], b2[e], b3[e]]
            ).reshape(1, H + H2 + O).astype(_nbf16)
        w_maps.append(m)

    # (token, slot) pairs per expert.
    tok_by_e = []
    wt_by_e = []
    for e in range(NCORES):
        tok, slot = np.nonzero(top_i == e)
        tok_by_e.append(tok)
        wt_by_e.append(top_p[tok, slot])

    out = np.zeros((B, O), np.float32)
    offset = [0] * NCORES
    last_result = None
    first_round = True
    while True:
        active = [e for e in range(NCORES) if offset[e] < len(tok_by_e[e])]
        if not active and last_result is not None:
            break
        # Round 1 uses the full-capacity NEFF. In the (never-observed) case
        # that an expert got more than C tokens, the leftovers run through a
        # small single-tile NEFF instead of paying for a full rerun.
        tws = TWS if first_round else OVERFLOW_TWS
        cap = sum(tws)
        nc = _get_nc(with_bias, tws)
        first_round = False
        in_maps = []
        chunks = []
        for e in range(NCORES):
            tok = tok_by_e[e][offset[e]:offset[e] + cap]
            chunks.append(tok)
            xt = np.zeros((P, KD, cap), _nbf16)
            if len(tok):
                # [n, D] -> [D, n] -> k-chunks [KD, P, n] -> [P, KD, n]
                xg = x[tok].astype(_nbf16).T.reshape(KD, P, len(tok))
                xt[:, :, :len(tok)] = xg.transpose(1, 0, 2)
            in_maps.append({"xt": np.ascontiguousarray(xt), **w_maps[e]})
        res = run_bass_kernel_spmd(
            nc, in_maps, core_ids=list(range(NCORES)), trace=trace
        )
        last_result = res
        for e in range(NCORES):
            tok = chunks[e]
            if len(tok) == 0:
                continue
            y = res.results[e]["out"][:, :len(tok)].T  # [n_e, O]
            w = wt_by_e[e][offset[e]:offset[e] + len(tok)]
            np.add.at(out, tok, w[:, None] * y)
            offset[e] += len(tok)
    return out, last_result


def kernel(x, Wr, br, W1, b1, W2, b2, W3, b3):
    x = np.asarray(x, np.float32)
    top_i, top_p = _route(x, np.asarray(Wr), np.asarray(br))
    out, _ = _run_rounds(
        x, top_i, top_p,
        np.asarray(W1), np.asarray(b1), np.asarray(W2), np.asarray(b2),
        np.asarray(W3), np.asarray(b3),
    )
    return out


def run_traced(x, Wr, br, W1, b1, W2, b2, W3, b3):
    """Like kernel() but returns (out, BassKernelResults) with profile info."""
    x = np.asarray(x, np.float32)
    top_i, top_p = _route(x, np.asarray(Wr), np.asarray(br))
    return _run_rounds(
        x, top_i, top_p,
        np.asarray(W1), np.asarray(b1), np.asarray(W2), np.asarray(b2),
        np.asarray(W3), np.asarray(b3),
        trace=True,
    )


# revision 55
# speedup vs baseline: 1.3573x; 1.0096x over previous
"""MoE (top-2 of 8 experts) Trainium2 kernel.

Strategy (expert-parallel over 8 NeuronCores):
  - Router (x @ Wr -> softmax -> top-2 -> renormalize) runs on host: it is
    ~0.1% of total FLOPs and produces the token->expert dispatch that defines
    the sharding itself.
  - Each core e receives the tokens routed to expert e (gathered, transposed
    to [D, C], zero-padded to capacity C) plus expert e's weights, and runs
    the 3-layer MLP fully on-device in a transposed dataflow:
        h1T = relu(W1^T x^T + b1)   [H,  C]
        h2T = relu(W2^T h1T + b2)   [H2, C]
        yT  = W3^T h2T + b3         [O,  C]
    All matmul contractions sit on the partition axis, so no on-chip
    transposes are needed anywhere.
  - Host combines per-expert outputs with the renormalized top-2 routing
    weights (scatter-add), exactly matching the reference's dense-combine
    semantics.
  - Matmuls run in bf16 with fp32 PSUM accumulation (measured ~4e-3 max
    relative error vs the fp32 reference).
  - If any expert receives more than C tokens (never observed; capacity is
    1.25x the expected per-expert load), the leftover tokens are processed in
    an additional run of the same NEFF - correctness never depends on C.
"""

import re as _re

import numpy as np
import ml_dtypes

import bass_rust as _bass_rust
import concourse.bass as bass
import concourse.mybir as mybir
import concourse.tile as tile
from concourse.bass_utils import run_bass_kernel_spmd


def _split_drain_and_barrier(self, tick_clock, wait_clock):
    """Replacement for TileContext._drain_and_barrier.

    The stock version hangs every outstanding proc semaphore wait on one
    Drain instruction; the walrus in this environment rejects any
    instruction carrying more than one sync wait. Emit the same waits as
    individual sync-engine wait_ge instructions (one wait each) before a
    clean drain instead.
    """
    ticks = [
        int(v)
        for v in _re.findall(r"\d+", repr(tick_clock.global_clock))
    ]
    for proc, sem in sorted(self.sems.allocated().items()):
        if proc < len(ticks) and ticks[proc] > 0:
            self.nc.sync.wait_ge(sem, _bass_rust.tick_to_sem(ticks[proc], proc))
    self.nc.sync.drain()

    self.nc.all_engine_barrier()
    assert self.sems is not None
    popped = self.nc._tile_sem_poison_stack.pop()
    assert popped is self._sem_poison
    self.nc.clear_and_free_semaphores(list(self.sems.allocated().values()))
    self.nc.all_engine_barrier()


tile.TileContext._drain_and_barrier = _split_drain_and_barrier

B, D, H, E, O, TOP_K = 8192, 1024, 2048, 8, 10, 2
H2 = H // 2
NCORES = 8
P = 128

TWS = [512, 512, 512, 512, 192]   # token tile widths (<=512 = one PSUM bank)
C = sum(TWS)      # per-expert token capacity (tokens, padded)
OVERFLOW_TWS = [512]              # small NEFF for the (never-seen) case of
                                  # an expert exceeding C tokens
KD = D // P       # 8   k-chunks for layer 1
MH = H // P       # 16  m-tiles for layer 1 / k-chunks for layer 2
MH2 = H2 // P     # 8   m-tiles for layer 2 / k-chunks for layer 3

BF16 = mybir.dt.bfloat16
F32 = mybir.dt.float32
_nbf16 = ml_dtypes.bfloat16


NW1 = KD * H          # w1 columns in the packed weight tile
NW2 = MH * H2         # w2 columns
NW3 = MH2 * O         # w3 columns
NWTOT = NW1 + NW2 + NW3
W1G = 4               # w1 arrives in this many m-major group DMAs


def _build_nc(with_bias: bool, tws) -> bass.Bass:
    cap = sum(tws)
    nc = bass.Bass()
    # Host pre-packs everything into the on-chip layout:
    #  xt   [128, KD, C]  — x gathered/transposed, k-chunks on axis 1
    #  w1/w2/w3 packed k-chunk-major: [128, KD*H] etc., bf16
    xt = nc.dram_tensor("xt", [P, KD, cap], BF16, kind="ExternalInput")
    w1d = nc.dram_tensor("w1p", [P, NW1], BF16, kind="ExternalInput")
    w2d = nc.dram_tensor("w2p", [P, NW2], BF16, kind="ExternalInput")
    w3d = nc.dram_tensor("w3p", [P, NW3], BF16, kind="ExternalInput")
    if with_bias:
        # biases as single-partition rows, pre-cast to bf16 on host:
        # cols [0,H) = b1, [H,H+H2) = b2, [H+H2,H+H2+O) = b3
        bias = nc.dram_tensor("bias", [1, H + H2 + O], BF16, kind="ExternalInput")
    out = nc.dram_tensor("out", [O, cap], F32, kind="ExternalOutput")

    relu_kw = dict(op0=mybir.AluOpType.max)

    with tile.TileContext(nc) as tc:
        with (
            tc.tile_pool(name="weights", bufs=1) as wpool,
            tc.tile_pool(name="xin", bufs=1) as xpool,
            tc.tile_pool(name="ps1", bufs=3, space="PSUM") as ps1pool,
            tc.tile_pool(name="ps2", bufs=3, space="PSUM") as ps2pool,
            tc.tile_pool(name="ps3", bufs=2, space="PSUM") as ps3pool,
            tc.tile_pool(name="acts", bufs=2) as apool,
        ):
            # w1 and the first x tile gate the first matmul group — give
            # each its own DMA (own HWDGE queue) so they stream in
            # parallel instead of behind one monolithic transfer. w1 is
            # further packed m-major and split into W1G group-DMAs with
            # separate tiles, so the m-loop's first groups start while the
            # rest of w1 is still in flight.
            MG = MH // W1G        # m-tiles per w1 group
            w1g_tiles = []
            last_w1_dma = None
            for g in range(W1G):
                w1g = wpool.tile([P, MG * KD * P], BF16, name=f"w1g{g}")
                last_w1_dma = nc.sync.dma_start(
                    w1g, w1d[:, g * MG * KD * P:(g + 1) * MG * KD * P])
                w1g_tiles.append(w1g)
            xsb_tiles = []
            off = 0
            for t, tw in enumerate(tws):
                xsb = xpool.tile([P, KD, tw], BF16, tag=f"x{t}")
                nc.sync.dma_start(xsb, xt[:, :, off:off + tw])
                xsb_tiles.append(xsb)
                off += tw
            w2sb = wpool.tile([P, NW2], BF16)
            nc.sync.dma_start(w2sb, w2d[:, :])
            w3sb = wpool.tile([P, NW3], BF16)
            nc.sync.dma_start(w3sb, w3d[:, :])

            def w1s(k, m):
                g, mm_ = divmod(m, MG)
                off = (mm_ * KD + k) * P
                return w1g_tiles[g][:, off:off + P]

            def w2s(k, m):
                off = k * H2 + m * P
                return w2sb[:, off:off + P]

            def w3s(k):
                off = k * O
                return w3sb[:, off:off + O]

            if with_bias:
                # Bias folded into each accumulation group as one extra K=1
                # matmul against a ones row: psum[m, n] += b[m] * 1. This
                # keeps bias handling entirely on the PE, so no evacuation
                # instruction ever needs a second semaphore wait.
                bsb = wpool.tile([1, H + H2 + O], BF16)
                nc.sync.dma_start(bsb, bias[:, :])
                ones = wpool.tile([1, max(tws)], BF16)
                nc.vector.memset(ones, 1.0)

            def bias_mm(ps, lo, hi, tw):
                if with_bias:
                    nc.tensor.matmul(
                        ps, bsb[:, lo:hi], ones[:, :tw], start=False, stop=True
                    )

            # 1-element DVE reads of the previous tile's activation buffers.
            # Slot reuse makes the first evacuation of a tile WAW-depend on
            # the previous tile's writes; the fence absorbs that own-engine
            # completion wait so no evacuation needs two semaphore waits
            # (the ISA wait slot fits only one).
            fence = wpool.tile([1, 4], BF16)
            prev = {}

            def dve_fence(key, ap):
                if key in prev:
                    nc.vector.tensor_copy(fence[:, 0:1], prev[key])
                prev[key] = ap



            tok_off = 0
            for t, tw in enumerate(tws):
                tok = slice(tok_off, tok_off + tw)
                tok_off += tw
                xsb = xsb_tiles[t]

                h1sb = apool.tile([P, MH, tw], BF16, tag="h1")
                dve_fence("h1", h1sb[0:1, 0, 0:1])
                for m in range(MH):
                    ps = ps1pool.tile([P, 512], F32, tag="ps1", name="ps1t")[:, :tw]
                    for k in range(KD):
                        nc.tensor.matmul(
                            ps,
                            w1s(k, m),
                            xsb[:, k, :],
                            start=(k == 0),
                            stop=(k == KD - 1) and not with_bias,
                        )
                    bias_mm(ps, m * P, (m + 1) * P, tw)
                    nc.vector.tensor_scalar(
                        h1sb[:, m, :], ps, 0.0, None, **relu_kw
                    )

                h2sb = apool.tile([P, MH2, tw], BF16, tag="h2")
                dve_fence("h2", h2sb[0:1, 0, 0:1])
                for m in range(MH2):
                    ps = ps2pool.tile([P, 512], F32, tag="ps2", name="ps2t")[:, :tw]
                    for k in range(MH):
                        nc.tensor.matmul(
                            ps,
                            w2s(k, m),
                            h1sb[:, k, :],
                            start=(k == 0),
                            stop=(k == MH - 1) and not with_bias,
                        )
                    bias_mm(ps, H + m * P, H + (m + 1) * P, tw)
                    nc.vector.tensor_scalar(
                        h2sb[:, m, :], ps, 0.0, None, **relu_kw
                    )

                ps3 = ps3pool.tile([P, 512], F32, tag="ps3", name="ps3t")[:O, :tw]
                for k in range(MH2):
                    nc.tensor.matmul(
                        ps3,
                        w3s(k),
                        h2sb[:, k, :],
                        start=(k == 0),
                        stop=(k == MH2 - 1) and not with_bias,
                    )
                bias_mm(ps3, H + H2, H + H2 + O, tw)
                # Per-tile SWDGE (gpsimd-issued) output transfer: overlaps
                # with later tiles' compute instead of sitting on the tail,
                # and doesn't occupy a HWDGE queue.
                osb = wpool.tile([O, tw], F32, name=f"osb{t}")
                nc.vector.tensor_copy(osb, ps3)
                nc.gpsimd.dma_start(out[:, tok], osb)
    return nc


_NC_CACHE: dict = {}


def _get_nc(with_bias: bool, tws) -> bass.Bass:
    key = (with_bias, tuple(tws))
    if key not in _NC_CACHE:
        _NC_CACHE[key] = _build_nc(with_bias, tws)
    return _NC_CACHE[key]


def _route(x, Wr, br):
    """Host router: softmax over logits, top-2, renormalized weights."""
    logits = x.astype(np.float32) @ Wr.astype(np.float32) + br.astype(np.float32)
    m = logits.max(axis=-1, keepdims=True)
    p = np.exp(logits - m)
    p /= p.sum(axis=-1, keepdims=True)
    top_i = np.argsort(-p, axis=-1, kind="stable")[:, :TOP_K]
    top_p = np.take_along_axis(p, top_i, axis=-1)
    top_p = top_p / top_p.sum(axis=-1, keepdims=True)
    return top_i.astype(np.int64), top_p.astype(np.float32)


def _run_rounds(x, top_i, top_p, W1, b1, W2, b2, W3, b3, trace=False):
    """Dispatch tokens to expert-owning cores, run the NEFF, combine."""
    with_bias = bool(np.any(b1) or np.any(b2) or np.any(b3))

    # Static per-core weight inputs, packed into the on-chip layout:
    # [128 partitions, k-chunk-major columns] per weight matrix.
    w_maps = []
    for e in range(NCORES):
        m = {
            # w1 m-major: [p, m, k, c] so the first m-groups lead the DMA
            "w1p": np.ascontiguousarray(
                W1[e].reshape(KD, P, MH, P).transpose(1, 2, 0, 3).reshape(P, NW1)
            ).astype(_nbf16),
            "w2p": np.ascontiguousarray(
                W2[e].reshape(MH, P, H2).transpose(1, 0, 2).reshape(P, NW2)
            ).astype(_nbf16),
            "w3p": np.ascontiguousarray(
                W3[e].reshape(MH2, P, O).transpose(1, 0, 2).reshape(P, NW3)
            ).astype(_nbf16),
        }
        if with_bias:
            m["bias"] = np.concatenate(
                [b1[e], b2[e], b3[e]]
            ).reshape(1, H + H2 + O).astype(_nbf16)
        w_maps.append(m)

    # (token, slot) pairs per expert.
    tok_by_e = []
    wt_by_e = []
    for e in range(NCORES):
        tok, slot = np.nonzero(top_i == e)
        tok_by_e.append(tok)
        wt_by_e.append(top_p[tok, slot])

    out = np.zeros((B, O), np.float32)
    offset = [0] * NCORES
    last_result = None
    first_round = True
    while True:
        active = [e for e in range(NCORES) if offset[e] < len(tok_by_e[e])]
        if not active and last_result is not None:
            break
        # Round 1 uses the full-capacity NEFF. In the (never-observed) case
        # that an expert got more than C tokens, the leftovers run through a
        # small single-tile NEFF instead of paying for a full rerun.
        tws = TWS if first_round else OVERFLOW_TWS
        cap = sum(tws)
        nc = _get_nc(with_bias, tws)
        first_round = False
        in_maps = []
        chunks = []
        for e in range(NCORES):
            tok = tok_by_e[e][offset[e]:offset[e] + cap]
            chunks.append(tok)
            xt = np.zeros((P, KD, cap), _nbf16)
            if len(tok):
                # [n, D] -> [D, n] -> k-chunks [KD, P, n] -> [P, KD, n]
                xg = x[tok].astype(_nbf16).T.reshape(KD, P, len(tok))
                xt[:, :, :len(tok)] = xg.transpose(1, 0, 2)
            in_maps.append({"xt": np.ascontiguousarray(xt), **w_maps[e]})
        res = run_bass_kernel_spmd(
            nc, in_maps, core_ids=list(range(NCORES)), trace=trace
        )
        last_result = res
        for e in range(NCORES):
            tok = chunks[e]
            if len(tok) == 0:
                continue
            y = res.results[e]["out"][:, :len(tok)].T  # [n_e, O]
            w = wt_by_e[e][offset[e]:offset[e] + len(tok)]
            np.add.at(out, tok, w[:, None] * y)
            offset[e] += len(tok)
    return out, last_result


def kernel(x, Wr, br, W1, b1, W2, b2, W3, b3):
    x = np.asarray(x, np.float32)
    top_i, top_p = _route(x, np.asarray(Wr), np.asarray(br))
    out, _ = _run_rounds(
        x, top_i, top_p,
        np.asarray(W1), np.asarray(b1), np.asarray(W2), np.asarray(b2),
        np.asarray(W3), np.asarray(b3),
    )
    return out


def run_traced(x, Wr, br, W1, b1, W2, b2, W3, b3):
    """Like kernel() but returns (out, BassKernelResults) with profile info."""
    x = np.asarray(x, np.float32)
    top_i, top_p = _route(x, np.asarray(Wr), np.asarray(br))
    return _run_rounds(
        x, top_i, top_p,
        np.asarray(W1), np.asarray(b1), np.asarray(W2), np.asarray(b2),
        np.asarray(W3), np.asarray(b3),
        trace=True,
    )
